# revision 3
# baseline (speedup 1.0000x reference)
"""Trainium2 Bass kernel for nn_GRUModel (segment-GRU encoder + 1-step GRU decoder).

Sharding: data-parallel over batch B: 8 cores x 16 batches each
(rows n = b_loc*64 + c, R=1024 rows/core). Weights replicated.

v2 layout: rows processed in two halves of C=512 (batches 0-7 / 8-15).
Encoder half-A runs all 16 steps, then half-B; decoder work for half-A
(ACT/DVE heavy, little PE) is interleaved into encoder half-B's steps so
the PE never idles behind it. Decoder half-B runs in the tail using
PE-assisted broadcast (identity-inject + select-matrix matmuls) to keep
the tail short.

State hT is [D(partitions), rows(free)]; recurrent matmul consumes what
the elementwise update produces -- no transposes. All matmuls bf16,
PSUM fp32. PSUM is drained eagerly (ACT Identity+bias) so banks recycle.
silu fused on ACT (AF.Silu). Decoder x-side gates (Wxd@pe + gbih) are
host-precomputed in fp32 (gxd for the DVE path, gxdT for the select path).

seq_last: encoder folds -last into a K=65 matmul row; output adds last
via DVE on a partition-replicated tile.
"""
import numpy as np
import ml_dtypes

import concourse.bass as bass
import concourse.bacc as bacc
import concourse.mybir as mybir
from concourse import tile
from concourse.bass_utils import run_bass_kernel_spmd

bf16 = ml_dtypes.bfloat16
F32 = mybir.dt.float32
BF16 = mybir.dt.bfloat16
AF = mybir.ActivationFunctionType
ALU = mybir.AluOpType

B, SEQ, ENC = 128, 1024, 64
D, SEG = 512, 64
SNX = SEQ // SEG          # 16
PRED = 512
SNY = PRED // SEG         # 8
NCORES = 8
BL = B // NCORES          # 16 batches per core
R = BL * ENC              # 1024 rows per core
C = R // 2                # 512 rows per half
BH = BL // 2              # 8 batches per half
KC = D // 128             # 4 contraction chunks
G3 = 3 * D                # 1536 gate dims
MC = G3 // 128            # 12 gate chunks

# bias column map (37 cols)
BC_EMB, BC_RZ, BC_HN, BC_XN, BC_RES, BC_GBHH, BC_PRED = 0, 4, 12, 16, 20, 24, 36

_PROGRAM = None


def _build_program():
    nc = bacc.Bacc("TRN2", target_bir_lowering=False, debug=False, num_devices=8)
    x_d = nc.dram_tensor("x", [BL, SEQ, ENC], F32, kind="ExternalInput")
    lastrow_d = nc.dram_tensor("lastrow", [1, R], F32, kind="ExternalInput")
    wemb_d = nc.dram_tensor("wemb", [65, D], BF16, kind="ExternalInput")
    wx_d = nc.dram_tensor("wx", [D, G3], BF16, kind="ExternalInput")
    wh_d = nc.dram_tensor("wh", [D, G3], BF16, kind="ExternalInput")
    wres_d = nc.dram_tensor("wres", [D, D], BF16, kind="ExternalInput")
    whd_d = nc.dram_tensor("whd", [D, G3], BF16, kind="ExternalInput")
    wpred_d = nc.dram_tensor("wpred", [D, SEG], BF16, kind="ExternalInput")
    gxd_d = nc.dram_tensor("gxd", [128, MC * SNY * ENC], BF16, kind="ExternalInput")
    gxdt_d = nc.dram_tensor("gxdt", [128, SNY * 6 * 128], BF16, kind="ExternalInput")
    ident_d = nc.dram_tensor("ident", [128, 128], BF16, kind="ExternalInput")
    selm_d = nc.dram_tensor("selm", [128, C], BF16, kind="ExternalInput")
    biases_d = nc.dram_tensor("biases", [128, 37], F32, kind="ExternalInput")
    o_d = nc.dram_tensor("o", [BL, PRED, ENC], F32, kind="ExternalOutput")

    with tile.TileContext(nc) as tc:
        with (
            tc.tile_pool(name="wp", bufs=1) as wp,
            tc.tile_pool(name="hpa", bufs=2) as hpa,
            tc.tile_pool(name="hpb", bufs=2) as hpb,
            tc.tile_pool(name="xsp", bufs=2) as xsp,
            tc.tile_pool(name="embp", bufs=2) as embp,
            tc.tile_pool(name="wk", bufs=1) as wk,
            tc.tile_pool(name="xnp", bufs=2) as xnp,
            tc.tile_pool(name="dg", bufs=1) as dg,
            tc.tile_pool(name="dwk", bufs=1) as dwk,
            tc.tile_pool(name="ytp", bufs=2) as ytp,
            tc.tile_pool(name="psum", bufs=8, space="PSUM") as pp,
        ):
            # ---- DMA loads, ordered by first use ----
            xsfA0 = xsp.tile([65, C], F32, tag="xsf")
            nc.sync.dma_start(
                xsfA0[0:64, :].rearrange("k (b c) -> k b c", b=BH),
                x_d[0:BH, 0:SEG, :].rearrange("b k c -> k b c"))
            nc.sync.dma_start(xsfA0[64:65, :], lastrow_d[:, 0:C])
            wemb = wp.tile([65, D], BF16, tag="wemb")
            nc.sync.dma_start(wemb[:], wemb_d[:])
            bia = wp.tile([128, 37], F32, tag="bia")
            nc.sync.dma_start(bia[:], biases_d[:])

            def wload(name, dram, width):
                t = wp.tile([128, KC * width], BF16, tag=name)
                nc.sync.dma_start(t[:].rearrange("p (kc j) -> p kc j", kc=KC),
                                  dram[:].rearrange("(kc p) j -> p kc j", p=128))
                return t

            wx = wload("wx", wx_d, G3)
            wh = wload("wh", wh_d, G3)
            wres = wload("wres", wres_d, D)
            whd = wload("whd", whd_d, G3)
            wpred = wload("wpred", wpred_d, SEG)
            gxd = wp.tile([128, MC * SNY * ENC], BF16, tag="gxd")
            nc.sync.dma_start(gxd[:], gxd_d[:])
            gxdt = wp.tile([128, SNY * 6 * 128], BF16, tag="gxdt")
            nc.sync.dma_start(gxdt[:], gxdt_d[:])
            ident = wp.tile([128, 128], BF16, tag="ident")
            nc.sync.dma_start(ident[:], ident_d[:])
            selm = wp.tile([128, C], BF16, tag="selm")
            nc.sync.dma_start(selm[:], selm_d[:])
            last64 = wp.tile([64, R], F32, tag="last64")
            nc.sync.dma_start(last64[:], lastrow_d[:].partition_broadcast(64))

            def wsl(w, kc, mc, width=G3):
                return w[:, kc * width + mc * 128: kc * width + mc * 128 + 128]

            halves = {}

            class H:
                pass

            for hname, b0, hp in (("A", 0, hpa), ("B", BH, hpb)):
                hh = H()
                hh.name, hh.b0, hh.hp = hname, b0, hp
                halves[hname] = hh

            def load_xs(hh, t):
                xsf = xsp.tile([65, C], F32, tag="xsf")
                nc.sync.dma_start(
                    xsf[0:64, :].rearrange("k (b c) -> k b c", b=BH),
                    x_d[hh.b0:hh.b0 + BH, t * SEG:(t + 1) * SEG, :]
                    .rearrange("b k c -> k b c"))
                nc.sync.dma_start(xsf[64:65, :],
                                  lastrow_d[:, hh.b0 * ENC: hh.b0 * ENC + C])
                return xsf

            def cast_xs(hh, xsf):
                xsb = xsp.tile([65, C], BF16, tag="xsb")
                nc.vector.tensor_copy(xsb[:], xsf[:])
                return xsb

            def emb_mms(hh, xsb):
                """embT = silu((x-last)@W_emb^T + b): [D, C] as 4 chunks."""
                embT = embp.tile([128, KC * C], BF16, tag="embT")
                for mc in range(KC):
                    ps = pp.tile([128, C], F32, tag="ps")
                    nc.tensor.matmul(ps[:], wemb[:, mc * 128:(mc + 1) * 128],
                                     xsb[:], start=True, stop=True)
                    nc.scalar.activation(embT[:, mc * C:(mc + 1) * C], ps[:],
                                         AF.Silu,
                                         bias=bia[:, BC_EMB + mc: BC_EMB + mc + 1])
                return embT

            def psxn_mms(hh, embT):
                """x-side n-gate pre-acts, drained to SBUF with bias bih_n."""
                xn = xnp.tile([128, KC * C], BF16, tag="xn")
                for mc in range(KC):
                    ps = pp.tile([128, C], F32, tag="ps")
                    for kc in range(KC):
                        nc.tensor.matmul(ps[:], wsl(wx, kc, 8 + mc),
                                         embT[:, kc * C:(kc + 1) * C],
                                         start=(kc == 0), stop=(kc == KC - 1))
                    nc.scalar.activation(xn[:, mc * C:(mc + 1) * C], ps[:],
                                         AF.Identity,
                                         bias=bia[:, BC_XN + mc: BC_XN + mc + 1])
                return xn

            def enc_step(hh, t):
                """One encoder step on half hh. Needs hh.embT (t), hh.xn (t),
                hh.hT (t, None at t=0). Produces them for t+1."""
                embT, xn, hT = hh.embT, hh.xn, hh.hT
                # prefetch x segment for t+1
                if t < SNX - 1:
                    xsf_next = load_xs(hh, t + 1)
                # A: rz groups
                rz = wk.tile([128, 8 * C], BF16, tag="rz")
                for mc in range(8):
                    ps = pp.tile([128, C], F32, tag="ps")
                    nk = KC if t > 0 else 0
                    for kc in range(KC):
                        nc.tensor.matmul(ps[:], wsl(wx, kc, mc),
                                         embT[:, kc * C:(kc + 1) * C],
                                         start=(kc == 0),
                                         stop=(nk == 0 and kc == KC - 1))
                    for kc in range(nk):
                        nc.tensor.matmul(ps[:], wsl(wh, kc, mc),
                                         hT[kc][:], start=False,
                                         stop=(kc == nk - 1))
                    nc.scalar.activation(rz[:, mc * C:(mc + 1) * C], ps[:],
                                         AF.Sigmoid,
                                         bias=bia[:, BC_RZ + mc: BC_RZ + mc + 1])
                # B: h-side n-gate; per-mc chain t1,t2,tanh,hc
                nsb = wk.tile([128, 4 * C], BF16, tag="nsb")
                hc = wk.tile([128, KC * C], BF16, tag="hc")
                t12 = wk.tile([128, 2 * C], BF16, tag="t12")
                hnsb = wk.tile([128, 4 * C], BF16, tag="hnsb")
                for mc in range(4):
                    rsl = rz[:, mc * C:(mc + 1) * C]
                    zsl = rz[:, (4 + mc) * C:(5 + mc) * C]
                    nsl = nsb[:, mc * C:(mc + 1) * C]
                    csl = hc[:, mc * C:(mc + 1) * C]
                    t1 = t12[:, 0:C]
                    t2 = t12[:, C:2 * C]
                    if t > 0:
                        ps = pp.tile([128, C], F32, tag="ps")
                        for kc in range(KC):
                            nc.tensor.matmul(ps[:], wsl(wh, kc, 8 + mc),
                                             hT[kc][:], start=(kc == 0),
                                             stop=(kc == KC - 1))
                        hsl = hnsb[:, mc * C:(mc + 1) * C]
                        nc.scalar.activation(hsl, ps[:], AF.Identity,
                                             bias=bia[:, BC_HN + mc: BC_HN + mc + 1])
                        nc.vector.tensor_tensor(t1, hsl, rsl, ALU.mult)
                    else:
                        nc.vector.tensor_scalar(
                            t1, rsl, bia[:, BC_HN + mc: BC_HN + mc + 1], None,
                            ALU.mult)
                    nc.vector.tensor_tensor(t2, xn[:, mc * C:(mc + 1) * C], t1,
                                            ALU.add)
                    nc.scalar.activation(nsl, t2, AF.Tanh)
                    # hc = n + z*(h - n)   (t=0: h=0 -> hc = n - z*n)
                    if t > 0:
                        nc.vector.tensor_tensor(csl, hT[mc][:], nsl, ALU.subtract)
                        nc.vector.tensor_tensor(csl, csl, zsl, ALU.mult)
                        nc.vector.tensor_tensor(csl, csl, nsl, ALU.add)
                    else:
                        nc.vector.tensor_tensor(csl, zsl, nsl, ALU.mult)
                        nc.vector.tensor_tensor(csl, nsl, csl, ALU.subtract)
                # C+D: next step's emb and x-side n-gate (PE cushion for res)
                if t < SNX - 1:
                    xsb_next = cast_xs(hh, xsf_next)
                    embT_next = emb_mms(hh, xsb_next)
                    xn_next = psxn_mms(hh, embT_next)
                # G: res projection, kc-outer so first MMs need only hc[0]
                psr = [pp.tile([128, C], F32, tag="ps", name=f"res{mc}")
                       for mc in range(KC)]
                for kc in range(KC):
                    for mc in range(KC):
                        nc.tensor.matmul(psr[mc][:], wsl(wres, kc, mc, D),
                                         hc[:, kc * C:(kc + 1) * C],
                                         start=(kc == 0), stop=(kc == KC - 1))
                hT_new = [hh.hp.tile([128, C], BF16, tag=f"h{hh.name}{i}",
                                     name=f"h{hh.name}{i}_{t}")
                          for i in range(KC)]
                for mc in range(KC):
                    nc.vector.scalar_tensor_tensor(
                        hT_new[mc][:], psr[mc][:],
                        bia[:, BC_RES + mc: BC_RES + mc + 1],
                        embT[:, mc * C:(mc + 1) * C], ALU.add, ALU.add)
                hh.hT = hT_new
                if t < SNX - 1:
                    hh.embT, hh.xn = embT_next, xn_next

            def ghd_mms(hh):
                """Decoder h-side gates + gbhh, drained to SBUF: [G3, C]."""
                ghd = dg.tile([128, MC * C], BF16, tag="ghd")
                for mc in range(MC):
                    ps = pp.tile([128, C], F32, tag="ps")
                    for kc in range(KC):
                        nc.tensor.matmul(ps[:], wsl(whd, kc, mc), hh.hT[kc][:],
                                         start=(kc == 0), stop=(kc == KC - 1))
                    nc.scalar.activation(ghd[:, mc * C:(mc + 1) * C], ps[:],
                                         AF.Identity,
                                         bias=bia[:, BC_GBHH + mc: BC_GBHH + mc + 1])
                return ghd

            def gxv(mc, s):
                """gxd view for fixed s, broadcast over b: [128, BH, ENC]."""
                v = gxd[:, mc * (SNY * ENC) + s * ENC: mc * (SNY * ENC) + (s + 1) * ENC]
                return v.unsqueeze(1).to_broadcast((128, BH, ENC))

            def dec_pred_store(hh, s, hy):
                ps = pp.tile([64, C], F32, tag="ps")
                for kc in range(KC):
                    nc.tensor.matmul(ps[:], wpred[:, kc * SEG:(kc + 1) * SEG],
                                     hy[:, kc * C:(kc + 1) * C],
                                     start=(kc == 0), stop=(kc == KC - 1))
                yt = ytp.tile([64, C], F32, tag="yt")
                nc.scalar.activation(yt[:], ps[:], AF.Identity,
                                     bias=bia[0:64, BC_PRED: BC_PRED + 1])
                nc.vector.tensor_tensor(
                    yt[:], yt[:], last64[:, hh.b0 * ENC: hh.b0 * ENC + C], ALU.add)
                nc.sync.dma_start(
                    o_d[hh.b0:hh.b0 + BH, s * SEG:(s + 1) * SEG, :]
                    .rearrange("b k c -> k b c"),
                    yt[:].rearrange("k (b c) -> k b c", b=BH))

            def dec_chunk_dve(hh, ghd, s):
                """Decoder step s on half hh, DVE-broadcast path (PE-light)."""
                rzd = dwk.tile([128, 8 * C], BF16, tag="rzd")
                for mc in range(8):
                    u = dwk.tile([128, C], BF16, tag="du")
                    nc.vector.tensor_tensor(
                        u[:].rearrange("p (b c) -> p b c", b=BH),
                        ghd[:, mc * C:(mc + 1) * C]
                        .rearrange("p (b c) -> p b c", b=BH),
                        gxv(mc, s), ALU.add)
                    nc.scalar.activation(rzd[:, mc * C:(mc + 1) * C], u[:],
                                         AF.Sigmoid)
                hy = dwk.tile([128, KC * C], BF16, tag="hy")
                for mc in range(4):
                    t1 = dwk.tile([128, C], BF16, tag="dt1")
                    nc.vector.tensor_tensor(t1[:], ghd[:, (8 + mc) * C:(9 + mc) * C],
                                            rzd[:, mc * C:(mc + 1) * C], ALU.mult)
                    t2 = dwk.tile([128, C], BF16, tag="dt2")
                    nc.vector.tensor_tensor(
                        t2[:].rearrange("p (b c) -> p b c", b=BH),
                        t1[:].rearrange("p (b c) -> p b c", b=BH),
                        gxv(8 + mc, s), ALU.add)
                    nd = dwk.tile([128, C], BF16, tag="dnd")
                    nc.scalar.activation(nd[:], t2[:], AF.Tanh)
                    zsl = rzd[:, (4 + mc) * C:(5 + mc) * C]
                    ysl = hy[:, mc * C:(mc + 1) * C]
                    eng = nc.vector if mc < 2 else nc.gpsimd
                    eng.tensor_tensor(ysl, hh.hT[mc][:], nd[:], ALU.subtract)
                    eng.tensor_tensor(ysl, ysl, zsl, ALU.mult)
                    eng.tensor_tensor(ysl, ysl, nd[:], ALU.add)
                dec_pred_store(hh, s, hy)

            def dec_chunk_sel(hh, ghd, s):
                """Decoder step s, select-matmul path (PE does the broadcast)."""
                rzd = dwk.tile([128, 8 * C], BF16, tag="rzd")
                for mc in range(8):
                    ps = pp.tile([128, C], F32, tag="ps")
                    nc.tensor.matmul(ps[:], ident[:], ghd[:, mc * C:(mc + 1) * C],
                                     start=True, stop=False)
                    hi = mc >= 6
                    nc.tensor.matmul(
                        ps[:],
                        gxdt[64 * hi:64 * hi + 64,
                             s * 768 + (mc % 6) * 128: s * 768 + (mc % 6) * 128 + 128],
                        selm[64 * hi:64 * hi + 64, :], start=False, stop=True)
                    nc.scalar.activation(rzd[:, mc * C:(mc + 1) * C], ps[:],
                                         AF.Sigmoid)
                hy = dwk.tile([128, KC * C], BF16, tag="hy")
                for mc in range(4):
                    mca = 8 + mc          # absolute gate chunk index
                    hi = mca >= 6
                    psn = pp.tile([128, C], F32, tag="ps")
                    nc.tensor.matmul(
                        psn[:],
                        gxdt[64 * hi:64 * hi + 64,
                             s * 768 + (mca % 6) * 128: s * 768 + (mca % 6) * 128 + 128],
                        selm[64 * hi:64 * hi + 64, :], start=True, stop=True)
                    t1 = dwk.tile([128, C], BF16, tag="dt1")
                    nc.vector.tensor_tensor(t1[:], ghd[:, mca * C:(mca + 1) * C],
                                            rzd[:, mc * C:(mc + 1) * C], ALU.mult)
                    t2 = dwk.tile([128, C], BF16, tag="dt2")
                    nc.vector.tensor_tensor(t2[:], psn[:], t1[:], ALU.add)
                    nd = dwk.tile([128, C], BF16, tag="dnd")
                    nc.scalar.activation(nd[:], t2[:], AF.Tanh)
                    zsl = rzd[:, (4 + mc) * C:(5 + mc) * C]
                    ysl = hy[:, mc * C:(mc + 1) * C]
                    eng = nc.vector if mc < 2 else nc.gpsimd
                    eng.tensor_tensor(ysl, hh.hT[mc][:], nd[:], ALU.subtract)
                    eng.tensor_tensor(ysl, ysl, zsl, ALU.mult)
                    eng.tensor_tensor(ysl, ysl, nd[:], ALU.add)
                dec_pred_store(hh, s, hy)

            # ================= schedule =================
            A, Bh = halves["A"], halves["B"]
            # prologue A
            xsbA0 = cast_xs(A, xsfA0)
            A.embT = emb_mms(A, xsbA0)
            A.xn = psxn_mms(A, A.embT)
            A.hT = None
            for t in range(SNX):
                enc_step(A, t)
            # prologue B
            xsfB0 = load_xs(Bh, 0)
            xsbB0 = cast_xs(Bh, xsfB0)
            Bh.embT = emb_mms(Bh, xsbB0)
            Bh.xn = psxn_mms(Bh, Bh.embT)
            Bh.hT = None
            ghdA = None
            for t in range(SNX):
                enc_step(Bh, t)
                if t == 7:
                    ghdA = ghd_mms(A)
                if t >= 8:
                    dec_chunk_dve(A, ghdA, t - 8)
            # tail: decoder half B via select path
            ghdB = ghd_mms(Bh)
            for s in range(SNY):
                dec_chunk_sel(Bh, ghdB, s)
    nc.finalize()
    return nc


def _prep_host(inputs):
    f = lambda a: np.ascontiguousarray(a, dtype=np.float32)
    bfc = lambda a: np.ascontiguousarray(a).astype(bf16)
    W_emb = f(inputs["W_emb"])                      # (D, SEG)
    wemb = np.zeros((65, D), np.float32)
    wemb[0:64, :] = W_emb.T
    wemb[64, :] = -W_emb.sum(axis=1)
    Wih, Whh = f(inputs["cell_Wih"]), f(inputs["cell_Whh"])
    bih, bhh = f(inputs["cell_bih"]), f(inputs["cell_bhh"])
    resW, resb = f(inputs["res_W"]), f(inputs["res_b"])
    gWih, gWhh = f(inputs["gru_Wih"]), f(inputs["gru_Whh"])
    gbih, gbhh = f(inputs["gru_bih"]), f(inputs["gru_bhh"])
    predW, predb = f(inputs["pred_W"]), f(inputs["pred_b"])
    pos_emb, channel_emb = f(inputs["pos_emb"]), f(inputs["channel_emb"])

    # pe columns j = s*64 + c: [pos[s] ; ch[c]]  -> (D, SNY*ENC)
    half = D // 2
    pe = np.zeros((D, SNY * ENC), np.float32)
    pe[0:half, :] = np.repeat(pos_emb.T, ENC, axis=1)
    pe[half:, :] = np.tile(channel_emb.T, (1, SNY))
    # decoder x-side gates with bias folded in: (G3, SNY*ENC), fp32 host math
    gx = gWih @ pe + gbih[:, None]
    # gxd: [128, mc * (s,c)]
    gxd = np.ascontiguousarray(
        gx.reshape(MC, 128, SNY * ENC).transpose(1, 0, 2).reshape(128, -1))
    # gxdt: [128, s*768 + (mc%6)*128 + g], partition = c + 64*(mc//6)
    gxdt = np.zeros((128, SNY * 6 * 128), np.float32)
    gxg = gx.reshape(MC, 128, SNY, ENC)             # [mc, g, s, c]
    for mc in range(MC):
        rowoff = 64 * (mc // 6)
        for s in range(SNY):
            gxdt[rowoff:rowoff + 64, s * 768 + (mc % 6) * 128:
                 s * 768 + (mc % 6) * 128 + 128] = gxg[mc, :, s, :].T
    ident = np.eye(128, dtype=np.float32)
    selm = np.zeros((128, C), np.float32)
    for c in range(64):
        selm[c, c::64] = 1.0
        selm[64 + c, c::64] = 1.0

    biases = np.zeros((128, 37), np.float32)

    def put(col, vec):
        nch = max(1, len(vec) // 128)
        for i in range(nch):
            seg = vec[i * 128:(i + 1) * 128]
            biases[0:len(seg), col + i] = seg

    put(BC_EMB, f(inputs["b_emb"]))
    put(BC_RZ, (bih + bhh)[0:1024])
    put(BC_HN, bhh[1024:1536])
    put(BC_XN, bih[1024:1536])
    put(BC_RES, resb)
    put(BC_GBHH, gbhh)
    put(BC_PRED, predb)

    return {
        "wemb": bfc(wemb),
        "wx": bfc(Wih.T), "wh": bfc(Whh.T), "wres": bfc(resW.T),
        "whd": bfc(gWhh.T), "wpred": bfc(predW.T),
        "gxd": bfc(gxd), "gxdt": bfc(gxdt),
        "ident": bfc(ident), "selm": bfc(selm),
        "biases": biases,
    }


def kernel(**inputs):
    global _PROGRAM
    if _PROGRAM is None:
        _PROGRAM = _build_program()
    nc = _PROGRAM
    shared = _prep_host(inputs)
    x = np.ascontiguousarray(inputs["x"], dtype=np.float32)
    in_maps = []
    for c in range(NCORES):
        xs = x[c * BL:(c + 1) * BL]
        m = dict(shared)
        m["x"] = xs
        m["lastrow"] = np.ascontiguousarray(xs[:, -1, :].reshape(1, R))
        in_maps.append(m)
    res = run_bass_kernel_spmd(nc, in_maps, list(range(NCORES)))
    out = np.concatenate([res.results[c]["o"] for c in range(NCORES)], axis=0)
    return out.astype(np.float32)
